# revision 31
# baseline (speedup 1.0000x reference)
"""Multi-head attention (B=8, N=2048, dim=64, heads=8) on 8 Trainium2 cores.

Sharding: batch-parallel - one batch element per NeuronCore, weights
replicated, no collectives. Per-core flash-style attention, fully
SBUF-resident.

v2: exp split across ACT/DVE/Pool engines (DVE+Pool via integer
Schraudolph exp-to-bf16-bits), fast reciprocal, bf16 output projection,
prefetch/setup/oproj share the st PSUM pool so PE never blocks on za.
"""
import sys

import numpy as np


def _ensure_path():
    try:
        import concourse  # noqa: F401
    except ImportError:
        for p in (
            "/opt/trn_rl_repo",
            "/root/.axon_site",
            "/root/.axon_site/_ro/trn_rl_repo",
            "/root/.axon_site/_ro/pypackages",
        ):
            if p not in sys.path:
                sys.path.append(p)


_ensure_path()

import concourse.bacc as bacc  # noqa: E402
import concourse.mybir as mybir  # noqa: E402
import concourse.tile as tile  # noqa: E402
from concourse.bass_utils import run_bass_kernel_spmd  # noqa: E402
from concourse.masks import make_identity  # noqa: E402

import os  # noqa: E402

DBG_NO_SCHRAUDOLPH = bool(os.environ.get("DBG_NO_SCHRAUDOLPH"))
DBG_SAFE_MISC = bool(os.environ.get("DBG_SAFE_MISC"))
DBG_SAFE_RECIP = DBG_SAFE_MISC or bool(os.environ.get("DBG_SAFE_RECIP"))
DBG_SAFE_MUL = DBG_SAFE_MISC or bool(os.environ.get("DBG_SAFE_MUL"))
DBG_SAFE_DMA = DBG_SAFE_MISC or bool(os.environ.get("DBG_SAFE_DMA"))

B, N, D, H = 8, 2048, 64, 8
P = 128
NT = N // P          # 16 j-tiles of 128
IC = N // 512        # 4 query chunks of 512
SCALE = float(D) ** -0.5
F32 = mybir.dt.float32
F32R = mybir.dt.float32r
BF16 = mybir.dt.bfloat16
I16 = mybir.dt.int16
I32 = mybir.dt.int32

# Schraudolph exp via bf16 bit pattern: bits = A16*s + B16.
# PSUM holds A16*s (Q pre-scaled by A16*SCALE); DVE/Pool add B16 and
# write int16 (truncation after +0.5 = round); ACT computes the exact
# exp(psum/A16). B16 includes -5.5 bits sawtooth centering so the
# mantissa-linear error is +-3% instead of [0, +6%].
A16 = 128.0 / float(np.log(2.0))        # 184.664
QPRE = A16 * SCALE                      # folded into W_q columns
EXPSCALE = 1.0 / A16
B16C = 16256.0 + 0.5 - 5.5

ALU = mybir.AluOpType
AF = mybir.ActivationFunctionType


def build_program(n_cores=B):
    nc = bacc.Bacc("TRN2", target_bir_lowering=False, debug=False,
                   num_devices=n_cores)
    x_d = nc.dram_tensor("x", [N, D], F32, kind="ExternalInput")
    wqkv_d = nc.dram_tensor("w_qkv", [D, 3 * H * D], F32, kind="ExternalInput")
    wout_d = nc.dram_tensor("w_out", [H * D, D], F32, kind="ExternalInput")
    bout_d = nc.dram_tensor("b_out", [D], F32, kind="ExternalInput")
    out_d = nc.dram_tensor("out", [N, D], F32, kind="ExternalOutput")

    with tile.TileContext(nc) as tc:
        with tc.tile_pool(name="const", bufs=1) as const:
            ident = const.tile([P, P], F32, tag="ident")
            make_identity(nc, ident[:])

            wsb = const.tile([D, 3 * H * D], F32R, tag="wqkv")
            nc.gpsimd.dma_start(wsb[:], wqkv_d.ap())
            # W_out in bf16 (cast during DMA on gpsimd)
            wout_sb = const.tile([P, 4, D], BF16, tag="wout")
            if DBG_SAFE_DMA:
                wout_f = const.tile([P, 4, D], F32, tag="woutf")
                nc.gpsimd.dma_start(
                    wout_f[:], wout_d.ap().rearrange("(t p) d -> p t d", p=P))
                nc.vector.tensor_copy(wout_sb[:], wout_f[:])
            else:
                nc.gpsimd.dma_start(
                    wout_sb[:],
                    wout_d.ap().rearrange("(t p) d -> p t d", p=P))
            b_row = const.tile([1, D], F32, tag="brow")
            nc.sync.dma_start(b_row[:], bout_d.ap().rearrange("(a d) -> a d", a=1))
            b_bc = const.tile([P, D], F32, tag="bbc")
            nc.gpsimd.partition_broadcast(b_bc[:], b_row[:])
            ones3 = const.tile([P, H, 1], BF16, tag="ones3")
            nc.gpsimd.memset(ones3[:], 1.0)
            # zeros operand for the stt clamp slot (op1 = max)
            zer = const.tile([P, 1024], BF16, tag="zer")
            nc.gpsimd.memset(zer[:], 0.0)
            # reciprocal bit-trick constant (exactly representable in f32);
            # [65,512] so row 64 shares the base partition of za's den row
            ctile = const.tile([65, 1024], F32, tag="ctile")
            nc.gpsimd.memset(ctile[:], 2129858432.0)

            xT = const.tile([D, N], F32R, tag="xT")
            # qk_sb[0..3]: Q^T head-pairs [128, N] (pre-scaled by A16*SCALE)
            # qk_sb[4..7]: K^T pairs
            qk_sb = [const.tile([P, N], BF16, tag=f"qk{i}", name=f"qk{i}")
                     for i in range(8)]
            # V~ per n-tile: [128, H, 65]; col 64 of each head is ones
            vt_sb = [const.tile([P, H, 65], BF16, tag=f"vt{t}", name=f"vt{t}")
                     for t in range(NT)]
            zT = [const.tile([P, N], BF16, tag=f"zT{i}", name=f"zT{i}")
                  for i in range(4)]

            with (
                tc.tile_pool(name="xin", bufs=1) as xpool,
                tc.tile_pool(name="spsum", bufs=3,
                             space=bacc.bass.MemorySpace.PSUM) as spsum,
                tc.tile_pool(name="zpsum", bufs=1,
                             space=bacc.bass.MemorySpace.PSUM) as zpsum,
                tc.tile_pool(name="es", bufs=6) as es_pool,
                tc.tile_pool(name="sm", bufs=4) as sm_pool,
                tc.tile_pool(name="outp", bufs=6) as outp,
            ):
                xall = xpool.tile([P, NT, D], F32, tag="xall")
                nc.sync.dma_start(
                    xall[:], x_d.ap().rearrange("(t p) d -> p t d", p=P))

                def st_tile():
                    # [128, 1024] = 2 PSUM banks; 3 bufs; shared by S-matmul
                    # chunks, setup, prefetch and oproj
                    return spsum.tile([P, 1024], F32, tag="st", name="st")

                def emit_qk(ct, icxs, copy_eng):
                    # Q tiles (ct < 4) get the A16*SCALE factor folded in
                    # during the PSUM->SBUF copy
                    w_sl = wsb[:, ct * P:(ct + 1) * P]
                    for icx in icxs:
                        mp = st_tile()
                        nc.tensor.matmul(
                            mp[0:P, 0:512], w_sl,
                            xT[:, icx * 512:(icx + 1) * 512],
                            start=True, stop=True)
                        dst = qk_sb[ct][:, icx * 512:(icx + 1) * 512]
                        if ct < 4:
                            copy_eng.scalar_tensor_tensor(
                                dst, mp[0:P, 0:512], QPRE, zer[:, 0:512],
                                op0=ALU.mult, op1=ALU.add)
                        else:
                            copy_eng.tensor_copy(dst, mp[0:P, 0:512])

                # ---- setup: transposes, Q/K for pair 0, all V
                for g in range(IC):
                    for t in range(4 * g, 4 * g + 4):
                        pp = st_tile()
                        nc.tensor.transpose(pp[0:D, 0:P], xall[:, t, :],
                                            ident[:])
                        nc.vector.tensor_copy(xT[:, t * P:(t + 1) * P],
                                              pp[0:D, 0:P])
                    emit_qk(4, [g], nc.vector)
                    emit_qk(0, [g], nc.vector)
                for t in range(NT):
                    mp = st_tile()
                    nc.tensor.matmul(
                        mp[0:P, 0:512], xT[:, t * P:(t + 1) * P],
                        wsb[:, 2 * H * D:3 * H * D],
                        start=True, stop=True)
                    nc.gpsimd.tensor_copy(vt_sb[t][:, :, 64:65], ones3[:])
                    nc.vector.tensor_copy(
                        vt_sb[t][:, :, 0:64],
                        mp[0:P, 0:512].rearrange("p (h d) -> p h d", h=H))

                # ---- main loop
                # chunk j covers both heads of a pair: st[:, 0:512] = head0,
                # st[:, 512:1024] = head1. exp split ACT 11 : DVE 5
                # (Pool cannot read PSUM).
                # AV runs AV_LAG chunks behind exp so the in-order PE queue
                # never blocks on an exp still in flight
                AV_LAG = 3
                pending = []  # [(es, j, za, hp), ...]

                def flush_av(nc, all_=False):
                    while pending and (all_ or len(pending) > AV_LAG):
                        es_p, j, za_p, hp_p = pending.pop(0)
                        for hh in (0, 1):
                            nc.tensor.matmul(
                                za_p[hh][:], vt_sb[j][:, 2 * hp_p + hh, :],
                                es_p[:, hh * 512:(hh + 1) * 512],
                                start=(j == 0), stop=(j == NT - 1),
                                skip_group_check=True)

                norm_pending = None  # (zus, hp, icx)

                def flush_norm(nc):
                    nonlocal norm_pending
                    if norm_pending is None:
                        return
                    zus_p, hp_p, icx_p = norm_pending
                    if True:
                        rc = sm_pool.tile([1, 1024], F32, tag="rc",
                                          name="rc")
                        if DBG_SAFE_RECIP:
                            nc.vector.reciprocal(rc[:], zus_p[64:65, :])
                        else:
                            # 1/den via bit-trick seed + one Newton step,
                            # plain DVE stt ops (no tables, no custom ops):
                            # seed_bits = C - bits(den); rc = sd*(2 - den*sd)
                            sd = sm_pool.tile([65, 1024], F32, tag="sd",
                                              name="sd")
                            nc.vector.scalar_tensor_tensor(
                                sd[64:65, :].bitcast(I32),
                                zus_p[64:65, :].bitcast(I32),
                                -1.0, ctile[64:65, :],
                                op0=ALU.mult, op1=ALU.add)
                            ee = sm_pool.tile([65, 1024], F32, tag="ee",
                                              name="ee")
                            nc.vector.scalar_tensor_tensor(
                                ee[64:65, :], zus_p[64:65, :], -1.0,
                                sd[64:65, :],
                                op0=ALU.mult, op1=ALU.mult)
                            nc.vector.scalar_tensor_tensor(
                                rc[:], ee[64:65, :], 2.0, sd[64:65, :],
                                op0=ALU.add, op1=ALU.mult)
                        bc = sm_pool.tile([64, 1024], F32, tag="bc",
                                          name="bc")
                        nc.gpsimd.partition_broadcast(bc[:], rc[:])
                        for hh in (0, 1):
                            nc.vector.tensor_mul(
                                zT[hp_p][hh * 64:hh * 64 + 64,
                                         icx_p * 512:(icx_p + 1) * 512],
                                zus_p[0:64, hh * 512:(hh + 1) * 512],
                                bc[:, hh * 512:(hh + 1) * 512])
                    norm_pending = None

                def emit_oproj(tiles):
                    for t in tiles:
                        op = st_tile()
                        for ct in range(4):
                            nc.tensor.matmul(
                                op[0:P, 0:D], zT[ct][:, t * P:(t + 1) * P],
                                wout_sb[:, ct, :],
                                start=(ct == 0), stop=(ct == 3),
                                skip_group_check=True)
                        ot = outp.tile([P, D], F32, tag="ot", name="ot")
                        nc.vector.tensor_add(ot[:], op[0:P, 0:D], b_bc[:])
                        nc.sync.dma_start(out_d.ap()[t * P:(t + 1) * P, :],
                                          ot[:])

                for hp in range(H // 2):
                    for icx in range(IC):
                        # next head-pair's Q/K prefetch happens mid-loop
                        # (j==5/j==11) so its DVE copy drains between chunks
                        qt = qk_sb[hp]
                        kt = qk_sb[4 + hp]
                        za = [zpsum.tile([65, 512], F32, tag="za0",
                                         name="za0", bufs=1),
                              zpsum.tile([65, 512], F32, tag="za1",
                                         name="za1", bufs=1)]
                        for j in range(NT):
                            st = st_tile()
                            es = es_pool.tile([P, 1024], BF16,
                                              tag="es", name="es")
                            for hh in (0, 1):
                                r0 = hh * 64
                                nc.tensor.matmul(
                                    st[:, hh * 512:(hh + 1) * 512],
                                    kt[r0:r0 + 64, j * P:(j + 1) * P],
                                    qt[r0:r0 + 64,
                                       icx * 512:(icx + 1) * 512],
                                    start=True, stop=True)
                            if j == 3:
                                # deferred normalization of the previous
                                # iteration, mid-loop so its DVE/Pool chain
                                # overlaps this iteration's chunk pipeline
                                flush_norm(nc)
                            if j == 5 and hp + 1 < H // 2:
                                emit_qk(4 + hp + 1, [icx], nc.vector)
                            if j == 11 and hp + 1 < H // 2:
                                emit_qk(hp + 1, [icx], nc.vector)
                            if j in (10, 13) and hp == 3 and icx >= 1:
                                # pair 3's icx-1 block is final: project
                                # those output tiles now (tail overlap),
                                # two tiles per burst to spread DVE/DMA load
                                t0 = 4 * (icx - 1) + (0 if j == 10 else 2)
                                emit_oproj(range(t0, t0 + 2))
                            if j in (2, 6, 10, 13) and not DBG_NO_SCHRAUDOLPH:
                                nc.vector.scalar_tensor_tensor(
                                    es[:].bitcast(I16), st[:], B16C,
                                    zer[:], op0=ALU.add, op1=ALU.max)
                            else:
                                nc.scalar.activation(
                                    es[:], st[:], AF.Exp, scale=EXPSCALE)
                            pending.append((es, j, za, hp))
                            flush_av(nc)
                        flush_av(nc, all_=True)
                        # stage za out of PSUM so banks free fast;
                        # normalization deferred one iteration
                        zu = sm_pool.tile([65, 1024], F32, tag="zu",
                                          name="zu")
                        nc.scalar.copy(zu[:, 0:512], za[0][:])
                        nc.vector.tensor_copy(zu[:, 512:1024], za[1][:])
                        norm_pending = (zu, hp, icx)
                flush_norm(nc)
                emit_oproj(range(12, NT))

    nc.compile()
    return nc


_PROG = None


def _get_program():
    global _PROG
    if _PROG is None:
        _PROG = build_program()
    return _PROG


def kernel(x, W_qkv, W_out, b_out):
    nc = _get_program()
    x = np.asarray(x, dtype=np.float32)
    wq = np.ascontiguousarray(np.asarray(W_qkv, dtype=np.float32))
    wo = np.ascontiguousarray(np.asarray(W_out, dtype=np.float32))
    bo = np.ascontiguousarray(np.asarray(b_out, dtype=np.float32))
    in_maps = [
        {"x": np.ascontiguousarray(x[i]), "w_qkv": wq, "w_out": wo,
         "b_out": bo}
        for i in range(B)
    ]
    res = run_bass_kernel_spmd(nc, in_maps, list(range(B)))
    return np.stack([res.results[i]["out"] for i in range(B)], axis=0)


# revision 36
# speedup vs baseline: 1.0286x; 1.0286x over previous
"""Multi-head attention (B=8, N=2048, dim=64, heads=8) on 8 Trainium2 cores.

Sharding: batch-parallel - one batch element per NeuronCore, weights
replicated, no collectives. Per-core flash-style attention, fully
SBUF-resident.

v2: exp split across ACT/DVE/Pool engines (DVE+Pool via integer
Schraudolph exp-to-bf16-bits), fast reciprocal, bf16 output projection,
prefetch/setup/oproj share the st PSUM pool so PE never blocks on za.
"""
import sys

import numpy as np


def _ensure_path():
    try:
        import concourse  # noqa: F401
    except ImportError:
        for p in (
            "/opt/trn_rl_repo",
            "/root/.axon_site",
            "/root/.axon_site/_ro/trn_rl_repo",
            "/root/.axon_site/_ro/pypackages",
        ):
            if p not in sys.path:
                sys.path.append(p)


_ensure_path()

import concourse.bacc as bacc  # noqa: E402
import concourse.mybir as mybir  # noqa: E402
import concourse.tile as tile  # noqa: E402
from concourse.bass_utils import run_bass_kernel_spmd  # noqa: E402
from concourse.masks import make_identity  # noqa: E402

import os  # noqa: E402

DBG_NO_SCHRAUDOLPH = bool(os.environ.get("DBG_NO_SCHRAUDOLPH"))
DBG_SAFE_MISC = bool(os.environ.get("DBG_SAFE_MISC"))
DBG_SAFE_RECIP = DBG_SAFE_MISC or bool(os.environ.get("DBG_SAFE_RECIP"))
DBG_SAFE_MUL = DBG_SAFE_MISC or bool(os.environ.get("DBG_SAFE_MUL"))
DBG_SAFE_DMA = DBG_SAFE_MISC or bool(os.environ.get("DBG_SAFE_DMA"))

B, N, D, H = 8, 2048, 64, 8
P = 128
NT = N // P          # 16 j-tiles of 128
IC = N // 512        # 4 query chunks of 512
SCALE = float(D) ** -0.5
F32 = mybir.dt.float32
F32R = mybir.dt.float32r
BF16 = mybir.dt.bfloat16
I16 = mybir.dt.int16
I32 = mybir.dt.int32

# Schraudolph exp via bf16 bit pattern: bits = A16*s + B16.
# PSUM holds A16*s (Q pre-scaled by A16*SCALE); DVE/Pool add B16 and
# write int16 (truncation after +0.5 = round); ACT computes the exact
# exp(psum/A16). B16 includes -5.5 bits sawtooth centering so the
# mantissa-linear error is +-3% instead of [0, +6%].
A16 = 128.0 / float(np.log(2.0))        # 184.664
QPRE = A16 * SCALE                      # folded into W_q columns
EXPSCALE = 1.0 / A16
B16C = 16256.0 + 0.5 - 5.5

ALU = mybir.AluOpType
AF = mybir.ActivationFunctionType


def build_program(n_cores=B):
    nc = bacc.Bacc("TRN2", target_bir_lowering=False, debug=False,
                   num_devices=n_cores)
    x_d = nc.dram_tensor("x", [N, D], F32, kind="ExternalInput")
    wqkv_d = nc.dram_tensor("w_qkv", [D, 3 * H * D], F32, kind="ExternalInput")
    wout_d = nc.dram_tensor("w_out", [H * D, D], F32, kind="ExternalInput")
    bout_d = nc.dram_tensor("b_out", [D], F32, kind="ExternalInput")
    out_d = nc.dram_tensor("out", [N, D], F32, kind="ExternalOutput")

    with tile.TileContext(nc) as tc:
        with tc.tile_pool(name="const", bufs=1) as const:
            ident = const.tile([P, P], F32, tag="ident")
            make_identity(nc, ident[:])

            wsb = const.tile([D, 3 * H * D], F32R, tag="wqkv")
            nc.gpsimd.dma_start(wsb[:], wqkv_d.ap())
            # W_out in bf16 (cast during DMA on gpsimd)
            wout_sb = const.tile([P, 4, D], BF16, tag="wout")
            if DBG_SAFE_DMA:
                wout_f = const.tile([P, 4, D], F32, tag="woutf")
                nc.gpsimd.dma_start(
                    wout_f[:], wout_d.ap().rearrange("(t p) d -> p t d", p=P))
                nc.vector.tensor_copy(wout_sb[:], wout_f[:])
            else:
                nc.gpsimd.dma_start(
                    wout_sb[:],
                    wout_d.ap().rearrange("(t p) d -> p t d", p=P))
            b_row = const.tile([1, D], F32, tag="brow")
            nc.sync.dma_start(b_row[:], bout_d.ap().rearrange("(a d) -> a d", a=1))
            b_bc = const.tile([P, D], F32, tag="bbc")
            nc.gpsimd.partition_broadcast(b_bc[:], b_row[:])
            ones3 = const.tile([P, H, 1], BF16, tag="ones3")
            nc.gpsimd.memset(ones3[:], 1.0)
            # zeros operand for the stt clamp slot (op1 = max)
            zer = const.tile([P, 1024], BF16, tag="zer")
            nc.gpsimd.memset(zer[:], 0.0)
            # reciprocal bit-trick constant (exactly representable in f32);
            # [65,512] so row 64 shares the base partition of za's den row
            ctile = const.tile([65, 1024], F32, tag="ctile")
            nc.gpsimd.memset(ctile[:], 2129858432.0)

            xT = const.tile([D, N], F32R, tag="xT")
            # qk_sb[0..3]: Q^T head-pairs [128, N] (pre-scaled by A16*SCALE)
            # qk_sb[4..7]: K^T pairs
            qk_sb = [const.tile([P, N], BF16, tag=f"qk{i}", name=f"qk{i}")
                     for i in range(8)]
            # V~ per n-tile: [128, H, 65]; col 64 of each head is ones
            vt_sb = [const.tile([P, H, 65], BF16, tag=f"vt{t}", name=f"vt{t}")
                     for t in range(NT)]
            zT = [const.tile([P, N], BF16, tag=f"zT{i}", name=f"zT{i}")
                  for i in range(4)]

            with (
                tc.tile_pool(name="xin", bufs=1) as xpool,
                tc.tile_pool(name="spsum", bufs=3,
                             space=bacc.bass.MemorySpace.PSUM) as spsum,
                tc.tile_pool(name="zpsum", bufs=1,
                             space=bacc.bass.MemorySpace.PSUM) as zpsum,
                tc.tile_pool(name="es", bufs=6) as es_pool,
                tc.tile_pool(name="sm", bufs=4) as sm_pool,
                tc.tile_pool(name="outp", bufs=6) as outp,
            ):
                xall = xpool.tile([P, NT, D], F32, tag="xall")
                nc.sync.dma_start(
                    xall[:], x_d.ap().rearrange("(t p) d -> p t d", p=P))

                def st_tile():
                    # [128, 1024] = 2 PSUM banks; 3 bufs; shared by S-matmul
                    # chunks, setup, prefetch and oproj
                    return spsum.tile([P, 1024], F32, tag="st", name="st")

                def emit_qk(ct, icxs, copy_eng):
                    # Q tiles (ct < 4) get the A16*SCALE factor folded in
                    # during the PSUM->SBUF copy
                    w_sl = wsb[:, ct * P:(ct + 1) * P]
                    for icx in icxs:
                        mp = st_tile()
                        nc.tensor.matmul(
                            mp[0:P, 0:512], w_sl,
                            xT[:, icx * 512:(icx + 1) * 512],
                            start=True, stop=True)
                        dst = qk_sb[ct][:, icx * 512:(icx + 1) * 512]
                        if ct < 4:
                            copy_eng.scalar_tensor_tensor(
                                dst, mp[0:P, 0:512], QPRE, zer[:, 0:512],
                                op0=ALU.mult, op1=ALU.add)
                        else:
                            copy_eng.tensor_copy(dst, mp[0:P, 0:512])

                # ---- setup: transposes, Q/K for pair 0, all V
                for g in range(IC):
                    for t in range(4 * g, 4 * g + 4):
                        pp = st_tile()
                        nc.tensor.transpose(pp[0:D, 0:P], xall[:, t, :],
                                            ident[:])
                        nc.vector.tensor_copy(xT[:, t * P:(t + 1) * P],
                                              pp[0:D, 0:P])
                    emit_qk(4, [g], nc.vector)
                    emit_qk(0, [g], nc.vector)
                for t in range(NT):
                    mp = st_tile()
                    nc.tensor.matmul(
                        mp[0:P, 0:512], xT[:, t * P:(t + 1) * P],
                        wsb[:, 2 * H * D:3 * H * D],
                        start=True, stop=True)
                    nc.gpsimd.tensor_copy(vt_sb[t][:, :, 64:65], ones3[:])
                    nc.vector.tensor_copy(
                        vt_sb[t][:, :, 0:64],
                        mp[0:P, 0:512].rearrange("p (h d) -> p h d", h=H))

                # ---- main loop
                # chunk j covers both heads of a pair: st[:, 0:512] = head0,
                # st[:, 512:1024] = head1. exp split ACT 11 : DVE 5
                # (Pool cannot read PSUM).
                # AV runs AV_LAG chunks behind exp so the in-order PE queue
                # never blocks on an exp still in flight
                AV_LAG = 3
                pending = []  # [(es, j, za, hp), ...]

                def flush_av(nc, all_=False):
                    while pending and (all_ or len(pending) > AV_LAG):
                        es_p, j, za_p, hp_p = pending.pop(0)
                        for hh in (0, 1):
                            nc.tensor.matmul(
                                za_p[hh][:], vt_sb[j][:, 2 * hp_p + hh, :],
                                es_p[:, hh * 512:(hh + 1) * 512],
                                start=(j == 0), stop=(j == NT - 1),
                                skip_group_check=True)

                norm_pending = None  # (zus, hp, icx)

                def flush_norm(nc):
                    nonlocal norm_pending
                    if norm_pending is None:
                        return
                    zus_p, hp_p, icx_p = norm_pending
                    if True:
                        rc = sm_pool.tile([1, 1024], F32, tag="rc",
                                          name="rc")
                        if DBG_SAFE_RECIP:
                            nc.vector.reciprocal(rc[:], zus_p[64:65, :])
                        else:
                            # 1/den via bit-trick seed + one Newton step,
                            # plain DVE stt ops (no tables, no custom ops):
                            # seed_bits = C - bits(den); rc = sd*(2 - den*sd)
                            sd = sm_pool.tile([65, 1024], F32, tag="sd",
                                              name="sd")
                            nc.vector.scalar_tensor_tensor(
                                sd[64:65, :].bitcast(I32),
                                zus_p[64:65, :].bitcast(I32),
                                -1.0, ctile[64:65, :],
                                op0=ALU.mult, op1=ALU.add)
                            ee = sm_pool.tile([65, 1024], F32, tag="ee",
                                              name="ee")
                            nc.vector.scalar_tensor_tensor(
                                ee[64:65, :], zus_p[64:65, :], -1.0,
                                sd[64:65, :],
                                op0=ALU.mult, op1=ALU.mult)
                            nc.vector.scalar_tensor_tensor(
                                rc[:], ee[64:65, :], 2.0, sd[64:65, :],
                                op0=ALU.add, op1=ALU.mult)
                        bc = sm_pool.tile([64, 1024], F32, tag="bc",
                                          name="bc")
                        nc.gpsimd.partition_broadcast(bc[:], rc[:])
                        for hh in (0, 1):
                            nc.vector.tensor_mul(
                                zT[hp_p][hh * 64:hh * 64 + 64,
                                         icx_p * 512:(icx_p + 1) * 512],
                                zus_p[0:64, hh * 512:(hh + 1) * 512],
                                bc[:, hh * 512:(hh + 1) * 512])
                    norm_pending = None

                def emit_oproj(tiles):
                    for t in tiles:
                        op = st_tile()
                        for ct in range(4):
                            nc.tensor.matmul(
                                op[0:P, 0:D], zT[ct][:, t * P:(t + 1) * P],
                                wout_sb[:, ct, :],
                                start=(ct == 0), stop=(ct == 3),
                                skip_group_check=True)
                        ot = outp.tile([P, D], F32, tag="ot", name="ot")
                        nc.vector.tensor_add(ot[:], op[0:P, 0:D], b_bc[:])
                        nc.sync.dma_start(out_d.ap()[t * P:(t + 1) * P, :],
                                          ot[:])

                for hp in range(H // 2):
                    for icx in range(IC):
                        # prefetch next head-pair's Q/K via the st pool
                        if hp + 1 < H // 2:
                            emit_qk(4 + hp + 1, [icx], nc.vector)
                            emit_qk(hp + 1, [icx], nc.vector)
                        qt = qk_sb[hp]
                        kt = qk_sb[4 + hp]
                        za = [zpsum.tile([65, 512], F32, tag="za0",
                                         name="za0", bufs=1),
                              zpsum.tile([65, 512], F32, tag="za1",
                                         name="za1", bufs=1)]
                        for j in range(NT):
                            st = st_tile()
                            es = es_pool.tile([P, 1024], BF16,
                                              tag="es", name="es")
                            for hh in (0, 1):
                                r0 = hh * 64
                                nc.tensor.matmul(
                                    st[:, hh * 512:(hh + 1) * 512],
                                    kt[r0:r0 + 64, j * P:(j + 1) * P],
                                    qt[r0:r0 + 64,
                                       icx * 512:(icx + 1) * 512],
                                    start=True, stop=True)
                            if j == 3:
                                # deferred normalization of the previous
                                # iteration, mid-loop so its DVE/Pool chain
                                # overlaps this iteration's chunk pipeline
                                flush_norm(nc)
                            if j in (10, 13) and hp == 3 and icx >= 1:
                                # pair 3's icx-1 block is final: project
                                # those output tiles now (tail overlap),
                                # two tiles per burst to spread DVE/DMA load
                                t0 = 4 * (icx - 1) + (0 if j == 10 else 2)
                                emit_oproj(range(t0, t0 + 2))
                            if j in (2, 6, 10, 13) and not DBG_NO_SCHRAUDOLPH:
                                nc.vector.scalar_tensor_tensor(
                                    es[:].bitcast(I16), st[:], B16C,
                                    zer[:], op0=ALU.add, op1=ALU.max)
                            else:
                                nc.scalar.activation(
                                    es[:], st[:], AF.Exp, scale=EXPSCALE)
                            pending.append((es, j, za, hp))
                            flush_av(nc)
                        flush_av(nc, all_=True)
                        # stage za out of PSUM so banks free fast;
                        # normalization deferred one iteration
                        zu = sm_pool.tile([65, 1024], F32, tag="zu",
                                          name="zu")
                        nc.scalar.copy(zu[:, 0:512], za[0][:])
                        nc.vector.tensor_copy(zu[:, 512:1024], za[1][:])
                        norm_pending = (zu, hp, icx)
                # final tiles 12..15: ct 0..2 need only zT[0..2]; run them
                # while the last norm chain is in flight. Accumulator slots
                # are bank-aligned (one PSUM bank each) because start=True
                # zeroes the enclosing bank region.
                op_a = st_tile()
                op_b = st_tile()
                slots = [(op_a, 0), (op_a, 512), (op_b, 0), (op_b, 512)]
                for ti, t in enumerate(range(12, NT)):
                    tl, off = slots[ti]
                    for ct in range(3):
                        nc.tensor.matmul(
                            tl[0:P, off:off + D],
                            zT[ct][:, t * P:(t + 1) * P],
                            wout_sb[:, ct, :],
                            start=(ct == 0), stop=False,
                            skip_group_check=True)
                flush_norm(nc)
                for ti, t in enumerate(range(12, NT)):
                    tl, off = slots[ti]
                    nc.tensor.matmul(
                        tl[0:P, off:off + D],
                        zT[3][:, t * P:(t + 1) * P],
                        wout_sb[:, 3, :],
                        start=False, stop=True, skip_group_check=True)
                    ot = outp.tile([P, D], F32, tag="ot", name="ot")
                    nc.vector.tensor_add(ot[:], tl[0:P, off:off + D],
                                         b_bc[:])
                    nc.sync.dma_start(out_d.ap()[t * P:(t + 1) * P, :],
                                      ot[:])

    nc.compile()
    return nc


_PROG = None


def _get_program():
    global _PROG
    if _PROG is None:
        _PROG = build_program()
    return _PROG


def kernel(x, W_qkv, W_out, b_out):
    nc = _get_program()
    x = np.asarray(x, dtype=np.float32)
    wq = np.ascontiguousarray(np.asarray(W_qkv, dtype=np.float32))
    wo = np.ascontiguousarray(np.asarray(W_out, dtype=np.float32))
    bo = np.ascontiguousarray(np.asarray(b_out, dtype=np.float32))
    in_maps = [
        {"x": np.ascontiguousarray(x[i]), "w_qkv": wq, "w_out": wo,
         "b_out": bo}
        for i in range(B)
    ]
    res = run_bass_kernel_spmd(nc, in_maps, list(range(B)))
    return np.stack([res.results[i]["out"] for i in range(B)], axis=0)
